# revision 1
# baseline (speedup 1.0000x reference)
"""Trainium2 Bass kernel for nn_CachedShapingFunctions (embedding_lookup).

out[b,t,w] = interp of lookup_table[:, w] at uniform-bucket position of
inputs[b,t,w].  Data-parallel over batch across 8 NeuronCores; the LUT is
replicated (as a host-prepared (value, delta) pair table per waveshaper).

Per-core pipeline (shard = [8192? no: 2 batches x 32768 t x 64 w] flattened
to [65536, 64]):
  - DMA in natural tiles, XBAR DMA-transpose (as 2x u16 planes) to
    waveshaper-on-partition layout [128 = 2 time-chunks x 64 w, 512 t]
  - DVE arithmetic: exact floor/clamp of the continuous bucket position
  - GPSIMD ap_gather of (value, delta) pairs from a per-partition table
  - strided extraction of the 1/16-dense gather output, interpolation
  - XBAR transpose back, DMA out
"""
import sys
import numpy as np

sys.path.insert(0, '/opt/trn_rl_repo')

import bass_rust
import concourse.bass as bass
import concourse.mybir as mybir
import concourse.tile as tile_mod
from concourse.tile import TileContext
from concourse.vector_clock import ScopedClock
from concourse import library_config

MIN_VALUE, MAX_VALUE = -3.0, 3.0
NB = 4096          # buckets
W = 64             # waveshapers
N_CORES = 8

# ---------------------------------------------------------------- patches --
# This walrus build accepts at most ONE sync-wait per instruction.  The Tile
# tail drain and scheduler can attach more; spill the excess onto nops.

_MAXW = 1

def _spill_waits(nc):
    for f in nc.m.functions:
        for bb in f.blocks:
            out = []
            for inst in list(bb.instructions):
                si = inst.sync_info
                if si is not None and len(si.on_wait) > _MAXW:
                    waits = list(si.on_wait)
                    spill = waits[:-_MAXW]
                    for i in range(0, len(spill), _MAXW):
                        nop = mybir.InstNoOp(
                            name=f"wspill_{inst.name}_{i}", ins=[], outs=[])
                        nop.engine = inst.engine
                        nop.sync_info = bass_rust.SyncInfo(
                            on_wait=spill[i:i + _MAXW], on_update=[])
                        out.append(nop)
                    inst.sync_info = bass_rust.SyncInfo(
                        on_wait=waits[-_MAXW:], on_update=list(si.on_update))
                out.append(inst)
            bb.instructions = out


def _patched_drain_and_barrier(self, tick_clock, wait_clock):
    nc = self.nc
    drain_inst = nc.sync.drain()
    wait_clock.add_sem_waits(
        drain_inst.ins, ScopedClock({None: tick_clock.global_clock}))
    si = drain_inst.ins.sync_info
    if si is not None and len(si.on_wait) > _MAXW:
        waits = list(si.on_wait)
        drain_inst.ins.sync_info = bass_rust.SyncInfo(
            on_wait=waits[:_MAXW], on_update=list(si.on_update))
        rest = waits[_MAXW:]
        for i in range(0, len(rest), _MAXW):
            nop = nc.sync.nop(hint="drain_wait_spill", nofuse=True)
            nop.ins.sync_info = bass_rust.SyncInfo(
                on_wait=rest[i:i + _MAXW], on_update=[])
    nc.all_engine_barrier()
    assert self.sems is not None
    popped = nc._tile_sem_poison_stack.pop()
    assert popped is self._sem_poison
    nc.clear_and_free_semaphores(list(self.sems.allocated().values()))
    nc.all_engine_barrier()


tile_mod.TileContext._drain_and_barrier = _patched_drain_and_barrier

# ----------------------------------------------------------------- kernel --

S = 512            # t-columns per transposed super-tile (per chunk)
TROWS = 2 * S      # natural t rows covered per super-tile (2 chunks)

F32 = mybir.dt.float32
I32 = mybir.dt.int32
I16 = mybir.dt.int16
U16 = mybir.dt.uint16


def build_kernel(n_rows):
    """n_rows: flattened time rows per core (65536 full scale)."""
    assert n_rows % TROWS == 0
    n_tiles = n_rows // TROWS
    nc = bass.Bass()
    x_d = nc.dram_tensor("x", [n_rows, W], F32, kind="ExternalInput")
    tbl_d = nc.dram_tensor("tbl", [128, NB * 2 + 16 + 128], F32, kind="ExternalInput")
    y_d = nc.dram_tensor("y", [n_rows, W], F32, kind="ExternalOutput")

    with TileContext(nc) as tc:
        with (
            tc.tile_pool(name="const", bufs=1) as cpool,
            tc.tile_pool(name="io", bufs=2) as iop,
            tc.tile_pool(name="tp", bufs=2) as tpp,
            tc.tile_pool(name="sc", bufs=1) as scp,
            tc.tile_pool(name="sc2", bufs=2) as scp2,
            tc.tile_pool(name="sp", bufs=2) as spp,
            tc.tile_pool(name="ps", bufs=2, space="PSUM") as psp,
        ):
            tbl = cpool.tile([128, NB * 2 + 16 + 128], F32)
            nc.sync.dma_start(tbl[:, :], tbl_d[:, :])
            nc.gpsimd.load_library(library_config.ap_gather)
            tbl3 = tbl[:, :NB * 2].rearrange("p (n d) -> p n d", d=2)
            mask = tbl[:, NB * 2: NB * 2 + 16]
            ident = tbl[:, NB * 2 + 16:]

            def emit_head(it):
                base = it * TROWS * W
                xnat = iop.tile([128, 8 * W], F32, tag="xnat")
                in_ap = bass.AP(x_d, base, [[W, 128], [128 * W, 8], [1, W]])
                nc.sync.dma_start(
                    xnat[:, :].rearrange("p (s w) -> p s w", s=8), in_ap)

                xT = tpp.tile([128, S], F32, tag="xT")
                for k in range(4):
                    pst = psp.tile([128, 128], F32, tag="psin")
                    nc.tensor.transpose(
                        pst[:, :], xnat[:, 128 * k: 128 * k + 128], ident)
                    nc.scalar.copy(xT[:, 128 * k: 128 * k + 128], pst[:, :])

                ic = scp.tile([128, S], F32, tag="ic")
                icc = scp.tile([128, S], F32, tag="icc")
                ili = scp.tile([128, S], I16, tag="ili")
                ilf = scp.tile([128, S], F32, tag="ilf")
                fd = scp.tile([128, S], F32, tag="fx")
                il2 = scp.tile([128, S], F32, tag="il2")
                dd = scp.tile([128, S], F32, tag="fx")
                fu = scp.tile([128, S], F32, tag="fu")
                il3 = scp.tile([128, S], F32, tag="ilf")
                ff = scp2.tile([128, S], F32, tag="ff")
                idx = scp2.tile([128, S], I16, tag="idx")

                A = mybir.AluOpType
                nc.vector.tensor_scalar(ic[:, :], xT[:, :], 3.0, 682.5, A.add, A.mult)
                nc.vector.tensor_scalar(icc[:, :], ic[:, :], 0.0, 4095.0, A.max, A.min)
                nc.vector.tensor_copy(ili[:, :], icc[:, :])
                nc.vector.tensor_copy(ilf[:, :], ili[:, :])
                nc.vector.tensor_tensor(fd[:, :], ilf[:, :], icc[:, :], A.is_gt)
                nc.vector.tensor_tensor(il2[:, :], ilf[:, :], fd[:, :], A.subtract)
                nc.vector.tensor_tensor(dd[:, :], icc[:, :], il2[:, :], A.subtract)
                nc.vector.tensor_scalar(fu[:, :], dd[:, :], 1.0, None, A.is_ge)
                nc.vector.tensor_tensor(il3[:, :], il2[:, :], fu[:, :], A.add)
                nc.vector.tensor_tensor(ff[:, :], ic[:, :], il3[:, :], A.subtract)
                nc.vector.tensor_copy(idx[:, :], il3[:, :])

                sparse = spp.tile([128, 16 * S * 2], F32, tag="sparse")
                sp3 = sparse[:, :].rearrange("p (n d) -> p n d", d=2)
                nc.gpsimd.ap_gather(sp3, tbl3, idx[:, :], channels=128,
                                    num_elems=NB, d=2, num_idxs=16 * S)
                return sparse, ff

            def emit_tail(it, sparse, ff):
                base = it * TROWS * W
                A = mybir.AluOpType
                sp3 = sparse[:, :].rearrange("p (n d) -> p n d", d=2)
                pairs = scp.tile([128, S * 2], F32, tag="pairs")
                pr3 = pairs[:, :].rearrange("p (n d) -> p n d", d=2)
                for r in range(16):
                    srcv = sparse[:, :].rearrange(
                        "p (n q) -> p n q", q=32)[:, :, 2 * r: 2 * r + 2]
                    mcol = mask[:, r: r + 1]
                    if r == 0:
                        nc.vector.tensor_scalar(
                            pr3, srcv, mcol, None, A.mult)
                    else:
                        nc.vector.scalar_tensor_tensor(
                            pr3, srcv, mcol, pr3, A.mult, A.add)

                outT = tpp.tile([128, S], F32, tag="outT")
                nc.vector.tensor_tensor(outT[:, :], pr3[:, :, 1], ff[:, :], A.mult)
                nc.vector.tensor_tensor(outT[:, :], outT[:, :], pr3[:, :, 0], A.add)

                onat = iop.tile([128, 8 * W], F32, tag="onat")
                for k in range(4):
                    pst = psp.tile([128, 128], F32, tag="psout")
                    nc.tensor.transpose(
                        pst[:, :], outT[:, 128 * k: 128 * k + 128], ident)
                    nc.scalar.copy(onat[:, 128 * k: 128 * k + 128], pst[:, :])

                out_ap = bass.AP(y_d, base, [[W, 128], [128 * W, 8], [1, W]])
                nc.sync.dma_start(
                    out_ap, onat[:, :].rearrange("p (s w) -> p s w", s=8))

            pending = None
            for it in range(n_tiles):
                sparse, ff = emit_head(it)
                if pending is not None:
                    emit_tail(*pending)
                pending = (it, sparse, ff)
            emit_tail(*pending)

    from concourse.library_overlay import lower_extended_insts
    lower_extended_insts(nc)
    _spill_waits(nc)
    return nc


def make_table(lookup_table):
    lut = np.asarray(lookup_table, dtype=np.float32)          # [4096, 64]
    vu = np.concatenate([lut[1:], lut[-1:]], axis=0)          # T[min(i+1,4095)]
    delta = vu - lut                                          # f32 exact
    pair = np.stack([lut, delta], axis=-1)                    # [4096, 64, 2]
    tblw = np.ascontiguousarray(pair.transpose(1, 0, 2)).reshape(W, NB * 2)
    tbl128 = np.concatenate([tblw, tblw], axis=0)             # [128, 8192]
    p = np.arange(128)
    m = (p[:, None] % 16 == np.arange(16)[None, :]).astype(np.float32)
    eye = np.eye(128, dtype=np.float32)
    return np.concatenate([tbl128, m, eye], axis=1)           # [128, 8336]


_CACHE = {}


def kernel(inputs, lookup_table):
    x = np.ascontiguousarray(np.asarray(inputs, dtype=np.float32))
    B, T, Wx = x.shape
    assert Wx == W
    per_core_b = B // N_CORES
    n_rows = per_core_b * T
    tbl = make_table(lookup_table)

    if n_rows not in _CACHE:
        _CACHE[n_rows] = build_kernel(n_rows)
    nc = _CACHE[n_rows]

    from concourse import bass_utils
    shards = x.reshape(N_CORES, n_rows, W)
    in_maps = [{"x": shards[c], "tbl": tbl} for c in range(N_CORES)]
    res = bass_utils.run_bass_kernel_spmd(
        nc, in_maps, core_ids=list(range(N_CORES)))
    out = np.stack([res.results[c]["y"] for c in range(N_CORES)], axis=0)
    return out.reshape(B, T, W)

